# revision 13
# baseline (speedup 1.0000x reference)
"""CfC head (mLSTM-style scan) Trainium2 kernel, v6.

Math (per timestep t, per (b,h)):
    i_t = exp(pre_i - n), f_t = exp(pre_f - n), o_t = exp(pre_o - n)
    g_t = sigmoid(pre_g); lam = sigmoid(pre_l)     (pre_* = w*xt + b)
    c   = f_t*c + i_t*g_t
    h   = (h + DT*o_t*sigmoid(c)) / (1 + DT*lam)
    n  += 0.01*(i_t + f_t + o_t - 3)
    y_t = h @ proj_w.T + proj_b

Device mapping: H=1024 sharded over 8 cores (128 h per core = partition dim);
free dim packs (batch-major, time-minor) blocks of TB steps, processed by the
X/gate machinery in 2048-wide half-blocks to respect the 8-bank PSUM budget.

n-handling: within a block, delta = n - n0 follows the affine scan
    dt_t = a_t*dt_{t-1} - 0.03   (dt = delta - 1, static additive tile)
    a_t = 1 - 0.01*P_t,  P_t = (Ei+Ef+Eo)_t * exp(-n0),
seeded per b-segment with dt_0 = 0.01*P_0 - 1.03. The gate correction is the
first-order e^{-delta} ~= 1 - delta = -dt, so
    ENd_t = e^{-n0}(1 - delta_{t-1}) = ENtile * dt_{t-1}
where ENtile = broadcast of -e^{-n0} over t, materialized by the PE:
ENtile[p,(b,t)] = sum_b' ENnT[b',p] * SEL[b',(b,t)] with SEL the static
b-selector and ENnT the PE-transposed carry. This keeps every full-size DVE
op in packed-fp16 2x mode (no broadcast access patterns, no STT).

Esum = Ei+Ef+Eo accumulates on the PE (identity matmuls) into the same PSUM
banks that carried the broadcast X (written by a ones-vector matmul from a
4KB staged x row), and is copied to fp16 SBUF by ACT.

c and h are exact affine scans given ENd:
    c_t = (Ef_t*ENd_t)*c_{t-1} + (Ei_t*g_t*ENd_t)
    h_t = L1_t*h_{t-1} + Eo_t*ENd_t*L1D_t*(Tc_t+1),  L1 = 1/(1+DT*lam)
L1 = (q-0.5)^2 + 0.75 via ACT Square+Identity (fp32: the h-scan decay error
is amplified ~200x); L1D = DT/2*L1 as a DVE tensor_scalar from Sq; sigmoids
use tanh so all ACT functions share the exp_and_others table. Carries track
EN32 = exp(-n) in fp32; n is never materialized.

y partials: per 128-col h-slab, slab-stationary matmul into [128,2] PSUM
columns, one 128-lane ACT copy, DMA out; host sums the 8 cores' partials.
"""

import os
from contextlib import ExitStack

import numpy as np

import concourse.bacc as bacc
import concourse.mybir as mybir
import concourse.tile as tile
from concourse.bass_utils import run_bass_kernel_spmd

AF = mybir.ActivationFunctionType
OP = mybir.AluOpType
F32 = mybir.dt.float32
F16 = mybir.dt.float16

B, S, H = 64, 2048, 1024
NCORES = 8
HC = H // NCORES
DT = 0.01

TB = int(os.environ.get("KERNEL_TB", "64"))  # timesteps per block
HW = 2048                                    # half-block width (PSUM-limited)
CCLAMP = 3.0e4

_cached = {}
_last_results = None


def build_program(s=S, tb=TB):
    nb = s // tb
    nfd = B * tb
    nslab = nfd // 128
    nhalf = nfd // HW              # X/gate passes per block
    tbh = tb // nhalf              # timesteps per half-block

    nc = bacc.Bacc(
        "TRN2", target_bir_lowering=False, debug=False, num_devices=NCORES
    )
    x_d = nc.dram_tensor("x", [B, s], F16, kind="ExternalInput").ap()
    wv_d = nc.dram_tensor("wv", [HC, 10], F32, kind="ExternalInput").ap()
    pj_d = nc.dram_tensor("projT", [HC, 2], F16, kind="ExternalInput").ap()
    en0_d = nc.dram_tensor("en0", [HC, 1], F32, kind="ExternalInput").ap()
    id_d = nc.dram_tensor("ident", [128, 128], F16, kind="ExternalInput").ap()
    sel_d = nc.dram_tensor("sel", [B, nfd], F16, kind="ExternalInput").ap()
    y_d = nc.dram_tensor("yout", [nb, 128, tb], F32, kind="ExternalOutput").ap()

    def r3(ap):  # [128, nfd] -> [128, B, tb]
        return ap.rearrange("p (b t) -> p b t", t=tb)

    with tile.TileContext(nc) as tc, ExitStack() as ctx:
        wp = ctx.enter_context(tc.tile_pool(name="w", bufs=1))
        pha = ctx.enter_context(tc.tile_pool(name="pha", bufs=2))
        chn = ctx.enter_context(tc.tile_pool(name="chn", bufs=1))
        sm = ctx.enter_context(tc.tile_pool(name="sm", bufs=2))
        pp = ctx.enter_context(tc.tile_pool(name="pp", bufs=1, space="PSUM"))
        pe = ctx.enter_context(tc.tile_pool(name="pe", bufs=1, space="PSUM"))
        pt = ctx.enter_context(tc.tile_pool(name="pt", bufs=1, space="PSUM"))
        pb = ctx.enter_context(tc.tile_pool(name="pb", bufs=2, space="PSUM"))

        wv = wp.tile([HC, 10], F32)
        nc.sync.dma_start(wv[:], wv_d)
        pj = wp.tile([HC, 2], F16)
        nc.sync.dma_start(pj[:], pj_d)
        en0t = wp.tile([HC, 1], F32)
        nc.sync.dma_start(en0t[:], en0_d)
        ident = wp.tile([128, 128], F16)
        nc.sync.dma_start(ident[:], id_d)
        sel = wp.tile([B, nfd], F16)
        nc.sync.dma_start(sel[:], sel_d)
        ones1 = wp.tile([1, 128], F16)
        nc.vector.memset(ones1[:], 1.0)

        rst = wp.tile([HC, nfd], F16)
        nc.vector.memset(rst[:], -0.03)
        bqm = wp.tile([HC, 1], F32)
        nc.vector.memset(bqm[:], DT / 2 - 0.5)
        b75 = wp.tile([HC, 1], F32)
        nc.vector.memset(b75[:], 0.75)
        bm1 = wp.tile([HC, 1], F32)
        nc.vector.memset(bm1[:], -1.0)
        b05 = wp.tile([HC, 1], F32)
        nc.vector.memset(b05[:], 0.5)
        bp1 = wp.tile([HC, 1], F32)
        nc.vector.memset(bp1[:], 1.0)

        # carries
        EN32 = wp.tile([HC, B], F32)
        nc.vector.memset(EN32[:], 1.0)
        nc.vector.tensor_scalar(EN32[:], EN32[:], en0t[:, 0:1], None, OP.mult)
        ENp = wp.tile([HC, B], F16)    # +exp(-n0) (for ENd col0)
        nc.scalar.copy(ENp[:], EN32[:])
        ENn = wp.tile([HC, B], F16)    # -exp(-n0) (transposed into ENtile)
        nc.scalar.mul(ENn[:], EN32[:], -1.0)
        cz = wp.tile([HC, B], F16)
        nc.vector.memset(cz[:], 0.0)
        hz = wp.tile([HC, B], F16)
        nc.vector.memset(hz[:], 0.0)
        Cc_v, Hc_v = cz[:], hz[:]

        ENtile = wp.tile([HC, nfd], F16)

        def make_entile():
            """ENtile[p,(b,t)] = ENn[p,b] via PE transpose + selector matmul."""
            psT = pt.tile([B, 128], F32, tag="T")
            nc.tensor.matmul(psT[:], ENn[:], ident[:], start=True, stop=True)
            ENnT = sm.tile([B, 128], F16, tag="ENnT")
            nc.scalar.copy(ENnT[:], psT[:])
            for c2 in range(nfd // 512):
                psB = pb.tile([128, 512], F32, tag="bc")
                sl = slice(512 * c2, 512 * (c2 + 1))
                nc.tensor.matmul(
                    psB[:], ENnT[:], sel[:, sl], start=True, stop=True
                )
                nc.scalar.copy(ENtile[:, sl], psB[:])

        make_entile()

        def stage_half(k, hf, gtiles):
            """x DMA + PE X-broadcast + gates + PE Esum + EsumH for half hf."""
            t0 = k * tb + hf * tbh
            hsl = slice(hf * HW, (hf + 1) * HW)
            xs = pha.tile([1, HW], F16, tag=f"xs{hf}")
            nc.sync.dma_start(
                xs[:].rearrange("p (b t) -> p b t", t=tbh),
                x_d[:, t0 : t0 + tbh].unsqueeze(0),
            )
            ps_xe = pe.tile([128, HW], F32, tag="xe")
            for c4 in range(HW // 512):
                sl = slice(512 * c4, 512 * (c4 + 1))
                nc.tensor.matmul(
                    ps_xe[:, sl], ones1[:], xs[:, sl], start=True, stop=True
                )
            Ei, Ef, Eo, Tg, Tl, EsumH = gtiles
            nc.scalar.activation(
                Ei[:, hsl], ps_xe[:], AF.Exp, bias=wv[:, 1:2], scale=wv[:, 0:1]
            )
            nc.scalar.activation(
                Ef[:, hsl], ps_xe[:], AF.Exp, bias=wv[:, 3:4], scale=wv[:, 2:3]
            )
            nc.scalar.activation(
                Eo[:, hsl], ps_xe[:], AF.Exp, bias=wv[:, 5:6], scale=wv[:, 4:5]
            )
            nc.scalar.activation(
                Tg[:, hsl], ps_xe[:], AF.Tanh, bias=wv[:, 7:8], scale=wv[:, 6:7]
            )
            # Tg <- g = 0.5*tanh + 0.5 (second ACT pass, in place)
            nc.scalar.activation(
                Tg[:, hsl], Tg[:, hsl], AF.Identity, bias=b05[:], scale=0.5
            )
            nc.scalar.activation(
                Tl[:, hsl], ps_xe[:], AF.Tanh, bias=wv[:, 9:10], scale=wv[:, 8:9]
            )
            for c4 in range(HW // 512):
                sl = slice(512 * c4, 512 * (c4 + 1))
                nc.tensor.matmul(
                    ps_xe[:, sl], ident[:], Ei[:, hsl][:, sl],
                    start=True, stop=False,
                )
                nc.tensor.matmul(
                    ps_xe[:, sl], ident[:], Ef[:, hsl][:, sl],
                    start=False, stop=False,
                )
                nc.tensor.matmul(
                    ps_xe[:, sl], ident[:], Eo[:, hsl][:, sl],
                    start=False, stop=True,
                )
            nc.scalar.copy(EsumH[:, hsl], ps_xe[:])

        def alloc_gtiles():
            Ei = pha.tile([128, nfd], F16, tag="Ei")
            Ef = pha.tile([128, nfd], F16, tag="Ef")
            Eo = pha.tile([128, nfd], F16, tag="Eo")
            Tg = pha.tile([128, nfd], F16, tag="Tg")
            Tl = chn.tile([128, nfd], F16, tag="Tl")
            EsumH = chn.tile([128, nfd], F16, tag="EsumH")
            return (Ei, Ef, Eo, Tg, Tl, EsumH)

        def stage_sq(gtiles):
            Tl = gtiles[4]
            Sq = chn.tile([128, nfd], F32, tag="Sq")
            nc.scalar.activation(
                Sq[:], Tl[:], AF.Square, bias=bqm[:], scale=DT / 2
            )
            L1 = chn.tile([128, nfd], F32, tag="L1")
            nc.scalar.activation(L1[:], Sq[:], AF.Identity, bias=b75[:])
            return Sq, L1

        # prologue: block 0 fully staged
        gt = alloc_gtiles()
        for hf in range(nhalf):
            stage_half(0, hf, gt)
        sq0, l10 = stage_sq(gt)
        cur = (gt, sq0, l10)

        def emit_y_mm(k, h):
            ps = pp.tile([128, tb], F32, tag="y")
            for j in range(nslab):
                nc.tensor.matmul(
                    ps[:, 2 * j : 2 * j + 2],
                    h[:, 128 * j : 128 * (j + 1)],
                    pj[:],
                    start=True,
                    stop=True,
                )
            return ps

        def emit_y_out(k, ps):
            ysb = sm.tile([128, tb], F32, tag="ysb")
            nc.scalar.copy(ysb[:], ps[:])
            nc.sync.dma_start(y_d[k], ysb[:])

        pend_y = None
        for k in range(nb):
            (Ei, Ef, Eo, Tg, Tl, EsumH), Sq, L1 = cur
            have_next = k + 1 < nb
            if have_next:
                ngt = alloc_gtiles()
                stage_half(k + 1, 0, ngt)
            if pend_y is not None:
                pend_ps = emit_y_mm(*pend_y)

            # ---- a (in place in EsumH), dt-scan ----
            a = EsumH
            nc.vector.tensor_mul(a[:], EsumH[:], ENtile[:])
            nc.vector.tensor_scalar(a[:], a[:], 0.01, 1.0, OP.mult, OP.add)
            nc.vector.tensor_scalar(
                r3(rst[:])[:, :, 0], r3(a[:])[:, :, 0], -1.0, -0.03,
                OP.mult, OP.add,
            )
            nc.vector.memset(r3(a[:])[:, :, 0], 0.0)
            dt = chn.tile([128, nfd], F16, tag="dt")
            nc.vector.tensor_tensor_scan(
                dt[:], a[:], rst[:], 0.0, OP.mult, OP.add
            )

            # ---- ENd (pre-update carries), then EN update + next ENtile ----
            Eend = sm.tile([HC, B], F32, tag="Eend")
            nc.scalar.activation(
                Eend[:], r3(dt[:])[:, :, tb - 1], AF.Exp, bias=bm1[:],
                scale=-1.0,
            )
            ENd = chn.tile([128, nfd], F16, tag="ENd")
            nc.vector.tensor_mul(
                ENd[:, 1:nfd], ENtile[:, 1:nfd], dt[:, 0 : nfd - 1]
            )
            nc.vector.tensor_copy(r3(ENd[:])[:, :, 0], ENp[:])
            nc.vector.tensor_mul(EN32[:], EN32[:], Eend[:])
            nc.scalar.copy(ENp[:], EN32[:])
            nc.scalar.mul(ENn[:], EN32[:], -1.0)
            if have_next:
                make_entile()

            # ---- c-scan: fc = Ef*ENd (in Ef), ic = Ei*ENd*g (in Ei) ----
            nc.vector.tensor_mul(Ef[:], Ef[:], ENd[:])
            nc.vector.tensor_mul(Ei[:], Ei[:], ENd[:])
            nc.vector.tensor_mul(Ei[:], Ei[:], Tg[:])
            t64 = sm.tile([HC, B], F16, tag="t64")
            nc.vector.tensor_mul(t64[:], r3(Ef[:])[:, :, 0], Cc_v)
            nc.vector.tensor_add(
                r3(Ei[:])[:, :, 0], r3(Ei[:])[:, :, 0], t64[:]
            )
            nc.vector.memset(r3(Ef[:])[:, :, 0], 0.0)
            c = chn.tile([128, nfd], F16, tag="c")
            nc.vector.tensor_tensor_scan(c[:], Ef[:], Ei[:], 0.0, OP.mult, OP.add)
            Ccl = sm.tile([HC, B], F16, tag="ccl")
            nc.vector.tensor_scalar_min(Ccl[:], r3(c[:])[:, :, tb - 1], CCLAMP)

            if have_next and nhalf > 1:
                stage_half(k + 1, 1, ngt)

            # ---- sigmoid(c) and Tc+1 on ACT ----
            Tc = chn.tile([128, nfd], F16, tag="Tc")
            nc.scalar.activation(Tc[:], c[:], AF.Tanh, scale=0.5)
            nc.scalar.activation(Tc[:], Tc[:], AF.Identity, bias=bp1[:])
            if pend_y is not None:
                emit_y_out(pend_y[0], pend_ps)
                pend_y = None

            # ---- h-scan: bh = Eo*ENd*L1D*(Tc+1) (in Eo) ----
            nc.vector.tensor_mul(Eo[:], Eo[:], ENd[:])
            L1D = chn.tile([128, nfd], F16, tag="L1D")
            nc.vector.tensor_scalar(
                L1D[:], Sq[:], DT / 2, 0.75 * DT / 2, OP.mult, OP.add
            )
            nc.vector.tensor_mul(Eo[:], Eo[:], L1D[:])
            t64b = sm.tile([HC, B], F32, tag="t64b")
            nc.vector.tensor_mul(t64b[:], r3(L1[:])[:, :, 0], Hc_v)
            nc.vector.tensor_mul(Eo[:], Eo[:], Tc[:])
            nc.vector.tensor_add(
                r3(Eo[:])[:, :, 0], r3(Eo[:])[:, :, 0], t64b[:]
            )
            nc.vector.memset(r3(L1[:])[:, :, 0], 0.0)

            if have_next:
                nsq, nl1 = stage_sq(ngt)

            h = chn.tile([128, nfd], F16, tag="h")
            nc.vector.tensor_tensor_scan(h[:], L1[:], Eo[:], 0.0, OP.mult, OP.add)

            Cc_v = Ccl[:]
            Hc_v = r3(h[:])[:, :, tb - 1]
            pend_y = (k, h)
            if not have_next:
                ps_l = emit_y_mm(*pend_y)
                emit_y_out(k, ps_l)
            else:
                cur = (ngt, nsq, nl1)

    nc.compile()
    return nc


def _get_program():
    key = (S, TB)
    if key not in _cached:
        _cached[key] = build_program(S, TB)
    return _cached[key]


def host_inputs(x_codes, Wi_w, Wi_b, Wf_w, Wf_b, Wo_w, Wo_b, Wg_w, Wg_b,
                Wl_w, Wl_b, proj_w, proj_b, n_init):
    """Fold input normalization into per-gate ACT scale/bias; shard over H."""
    f = lambda v: np.asarray(v, np.float32)
    cols = []
    for (w, b) in ((Wi_w, Wi_b), (Wf_w, Wf_b), (Wo_w, Wo_b)):
        cols += [f(w) / 100.0, f(b) - 0.65 * f(w)]
    for (w, b) in ((Wg_w, Wg_b), (Wl_w, Wl_b)):
        cols += [f(w) / 200.0, (f(b) - 0.65 * f(w)) / 2.0]
    wv_full = np.stack(cols, axis=1).astype(np.float32)  # [H, 10]
    x = np.ascontiguousarray(f(x_codes)).astype(np.float16)
    pw = f(proj_w)
    en0 = np.exp(-f(n_init))
    ident = np.eye(128, dtype=np.float16)
    nfd = B * TB
    sel = np.zeros((B, nfd), np.float16)
    for b in range(B):
        sel[b, b * TB : (b + 1) * TB] = 1.0
    maps = []
    for k in range(NCORES):
        hs = slice(k * HC, (k + 1) * HC)
        maps.append({
            "x": x,
            "wv": np.ascontiguousarray(wv_full[hs]),
            "projT": np.ascontiguousarray(pw[:, hs].T.astype(np.float16)),
            "en0": np.ascontiguousarray(en0[hs].reshape(HC, 1)),
            "ident": ident,
            "sel": sel,
        })
    return maps


def assemble_output(results, proj_b, s=S, tb=TB):
    nb = s // tb
    nslab = (B * tb) // 128
    bper = 128 // tb if tb <= 128 else 1
    y = np.zeros((B, s, 2), np.float64)
    for k in range(NCORES):
        yc = np.asarray(results[k]["yout"], np.float64)
        ycr = yc.reshape(nb, bper, tb, nslab, 2)
        y += np.transpose(ycr, (3, 1, 0, 2, 4)).reshape(B, s, 2)
    y += np.asarray(proj_b, np.float64)[None, None, :]
    return y.astype(np.float32)


def kernel(**inputs):
    global _last_results
    nc = _get_program()
    maps = host_inputs(**inputs)
    res = run_bass_kernel_spmd(
        nc, maps, list(range(NCORES)),
        trace=bool(os.environ.get("KTRACE")),
        tmpdir=os.environ.get("KTRACE_DIR") or None,
    )
    _last_results = res
    return assemble_output(res.results, inputs["proj_b"])


# revision 15
# speedup vs baseline: 1.1825x; 1.1825x over previous
"""CfC head (mLSTM-style scan) Trainium2 kernel, v4.

Math (per timestep t, per (b,h)):
    i_t = exp(pre_i - n), f_t = exp(pre_f - n), o_t = exp(pre_o - n)
    g_t = sigmoid(pre_g); lam = sigmoid(pre_l)     (pre_* = w*xt + b)
    c   = f_t*c + i_t*g_t
    h   = (h + DT*o_t*sigmoid(c)) / (1 + DT*lam)
    n  += 0.01*(i_t + f_t + o_t - 3)
    y_t = h @ proj_w.T + proj_b

Device mapping: H=1024 sharded over 8 cores (128 h per core = partition dim);
free dim packs (batch-major, time-minor) blocks of TB=32 steps.

n-handling: within a block, delta = n - n0 follows the affine scan
    dt_t = a_t*dt_{t-1} - 0.03   (dt = delta - 1, so the additive operand is
    the STATIC tile -0.03; only each b-segment's first column is data),
    a_t = 1 - 0.01*P_t,  P_t = (Ei+Ef+Eo)_t * exp(-n0),
seeded per segment with dt_0 = 0.01*P_0 - 1.03. The gate correction is the
first-order e^{-delta} ~= 1 - delta = -dt:
    ENd_t = e^{-n0}(1 - delta_{t-1}) = (-e^{-n0}) * dt_{t-1}
i.e. one broadcast multiply - no exp() on the ACT engine for the correction.

Esum = Ei+Ef+Eo accumulates on the PE (identity matmuls) into the same PSUM
banks that carried the broadcast X. X itself is broadcast across partitions
by the PE (ones-vector stationary matmul from a 4KB staged x row) instead of
a 512KB broadcast DMA; the gates read X straight from PSUM.

c and h are exact affine scans given ENd:
    c_t = (Ef_t*ENd_t)*c_{t-1} + (Ei_t*g_t*ENd_t)
    h_t = L1_t*h_{t-1} + Eo_t*ENd_t*L1D_t*(Tc_t+1),  L1 = 1/(1+DT*lam)
L1 = (q-0.5)^2 + 0.75 via ACT Square+Identity (fp32: the h-scan decay error
is amplified ~200x so it must stay fp32); L1D = DT/2*L1 as a DVE
tensor_scalar from Sq; g = 0.5*tanh+0.5 and Tc+1 are second ACT passes
(sigmoids use tanh so all ACT functions share the exp_and_others table).
Carries track EN32 = exp(-n) directly in fp32; n is never materialized.

Emission interleaves next-block stage-A ACT work with this block's DVE chain
so the DVE (the bottleneck engine at ~97% busy) never starves.

y partials: per 128-col h-slab, slab-stationary matmul into [128,2] PSUM
columns, one 128-lane ACT copy, DMA out; host sums the 8 cores' partials.
"""

import os
from contextlib import ExitStack

import numpy as np

import concourse.bacc as bacc
import concourse.mybir as mybir
import concourse.tile as tile
from concourse.bass_utils import run_bass_kernel_spmd

AF = mybir.ActivationFunctionType
OP = mybir.AluOpType
F32 = mybir.dt.float32
F16 = mybir.dt.float16

B, S, H = 64, 2048, 1024
NCORES = 8
HC = H // NCORES
DT = 0.01

TB = int(os.environ.get("KERNEL_TB", "32"))
CCLAMP = 3.0e4

_cached = {}
_last_results = None


def build_program(s=S, tb=TB):
    nb = s // tb
    nfd = B * tb
    nslab = nfd // 128

    nc = bacc.Bacc(
        "TRN2", target_bir_lowering=False, debug=False, num_devices=NCORES
    )
    x_d = nc.dram_tensor("x", [B, s], F16, kind="ExternalInput").ap()
    wv_d = nc.dram_tensor("wv", [HC, 10], F32, kind="ExternalInput").ap()
    pj_d = nc.dram_tensor("projT", [HC, 2], F16, kind="ExternalInput").ap()
    en0_d = nc.dram_tensor("en0", [HC, 1], F32, kind="ExternalInput").ap()
    id_d = nc.dram_tensor("ident", [128, 128], F16, kind="ExternalInput").ap()
    y_d = nc.dram_tensor("yout", [nb, 128, tb], F32, kind="ExternalOutput").ap()

    def r3(ap):  # [128, nfd] -> [128, B, tb]
        return ap.rearrange("p (b t) -> p b t", t=tb)

    with tile.TileContext(nc) as tc, ExitStack() as ctx:
        wp = ctx.enter_context(tc.tile_pool(name="w", bufs=1))
        pha = ctx.enter_context(tc.tile_pool(name="pha", bufs=2))
        chn = ctx.enter_context(tc.tile_pool(name="chn", bufs=1))
        sm = ctx.enter_context(tc.tile_pool(name="sm", bufs=2))
        pp = ctx.enter_context(tc.tile_pool(name="pp", bufs=2, space="PSUM"))
        pe = ctx.enter_context(tc.tile_pool(name="pe", bufs=1, space="PSUM"))

        wv = wp.tile([HC, 10], F32)
        nc.sync.dma_start(wv[:], wv_d)
        pj = wp.tile([HC, 2], F16)
        nc.sync.dma_start(pj[:], pj_d)
        en0t = wp.tile([HC, 1], F32)
        nc.sync.dma_start(en0t[:], en0_d)
        ident = wp.tile([128, 128], F16)
        nc.sync.dma_start(ident[:], id_d)
        ones1 = wp.tile([1, 128], F16)
        nc.vector.memset(ones1[:], 1.0)

        rst = wp.tile([HC, nfd], F16)
        nc.vector.memset(rst[:], -0.03)
        bqm = wp.tile([HC, 1], F32)
        nc.vector.memset(bqm[:], DT / 2 - 0.5)
        b75 = wp.tile([HC, 1], F32)
        nc.vector.memset(b75[:], 0.75)
        bm1 = wp.tile([HC, 1], F32)
        nc.vector.memset(bm1[:], -1.0)
        b05 = wp.tile([HC, 1], F32)
        nc.vector.memset(b05[:], 0.5)
        bp1 = wp.tile([HC, 1], F32)
        nc.vector.memset(bp1[:], 1.0)
        bld = wp.tile([HC, 1], F32)
        nc.vector.memset(bld[:], 0.75 * DT / 2)

        # carries
        EN32 = wp.tile([HC, B], F32)
        nc.vector.memset(EN32[:], 1.0)
        nc.vector.tensor_scalar(EN32[:], EN32[:], en0t[:, 0:1], None, OP.mult)
        ENp = wp.tile([HC, B], F16)
        nc.scalar.copy(ENp[:], EN32[:])
        ENn = wp.tile([HC, B], F16)
        nc.scalar.mul(ENn[:], EN32[:], -1.0)
        cz = wp.tile([HC, B], F16)
        nc.vector.memset(cz[:], 0.0)
        hz = wp.tile([HC, B], F16)
        nc.vector.memset(hz[:], 0.0)
        Cc_v, Hc_v = cz[:], hz[:]

        def stage_x(k):
            """x staging DMA + PE broadcast into the shared XE PSUM tile."""
            xs = pha.tile([1, nfd], F16, tag="xs")
            nc.sync.dma_start(
                xs[:].rearrange("p (b t) -> p b t", t=tb),
                x_d[:, k * tb : (k + 1) * tb].unsqueeze(0),
            )
            ps_xe = pe.tile([128, nfd], F32, tag="xe")
            for c4 in range(nfd // 512):
                sl = slice(512 * c4, 512 * (c4 + 1))
                nc.tensor.matmul(
                    ps_xe[:, sl], ones1[:], xs[:, sl], start=True, stop=True
                )
            return ps_xe

        def stage_gates1(k, ps_xe):
            """first gate batch: 3 exps + g (tanh + affine) from PSUM X."""
            Ei = pha.tile([128, nfd], F16, tag="Ei")
            nc.scalar.activation(
                Ei[:], ps_xe[:], AF.Exp, bias=wv[:, 1:2], scale=wv[:, 0:1]
            )
            Ef = pha.tile([128, nfd], F16, tag="Ef")
            nc.scalar.activation(
                Ef[:], ps_xe[:], AF.Exp, bias=wv[:, 3:4], scale=wv[:, 2:3]
            )
            Eo = pha.tile([128, nfd], F16, tag="Eo")
            nc.scalar.activation(
                Eo[:], ps_xe[:], AF.Exp, bias=wv[:, 5:6], scale=wv[:, 4:5]
            )
            Tg = pha.tile([128, nfd], F16, tag="Tg")
            nc.scalar.activation(
                Tg[:], ps_xe[:], AF.Tanh, bias=wv[:, 7:8], scale=wv[:, 6:7]
            )
            nc.scalar.activation(
                Tg[:], Tg[:], AF.Identity, bias=b05[:], scale=0.5
            )
            return Ei, Ef, Eo, Tg

        def stage_gates2(k, ps_xe):
            """second gate batch: Tl, Sq."""
            Tl = pha.tile([128, nfd], F16, tag="Tl")
            nc.scalar.activation(
                Tl[:], ps_xe[:], AF.Tanh, bias=wv[:, 9:10], scale=wv[:, 8:9]
            )
            Sq = pha.tile([128, nfd], F32, tag="Sq")
            nc.scalar.activation(
                Sq[:], Tl[:], AF.Square, bias=bqm[:], scale=DT / 2
            )
            return Tl, Sq

        def stage_esum(k, ps_xe, Ei, Ef, Eo):
            """Esum accumulates into the XE banks after the gates read X."""
            for c4 in range(nfd // 512):
                sl = slice(512 * c4, 512 * (c4 + 1))
                nc.tensor.matmul(
                    ps_xe[:, sl], ident[:], Ei[:, sl], start=True, stop=False
                )
                nc.tensor.matmul(
                    ps_xe[:, sl], ident[:], Ef[:, sl], start=False, stop=False
                )
                nc.tensor.matmul(
                    ps_xe[:, sl], ident[:], Eo[:, sl], start=False, stop=True
                )
            return ps_xe

        def stage_l1(k, Sq):
            L1 = pha.tile([128, nfd], F32, tag="L1")
            nc.scalar.activation(L1[:], Sq[:], AF.Identity, bias=b75[:])
            return L1

        # prologue
        ps_xe0 = stage_x(0)
        g1 = stage_gates1(0, ps_xe0)
        g2 = stage_gates2(0, ps_xe0)
        es0 = stage_esum(0, ps_xe0, g1[0], g1[1], g1[2])
        l10 = stage_l1(0, g2[1])
        tiles = (g1[0], g1[1], g1[2], g1[3], g2[0], g2[1], l10, es0)
        for k in range(nb):
            Ei, Ef, Eo, Tg, Tl, Sq, L1, ps_es = tiles
            have_next = k + 1 < nb
            if have_next:
                nxt_xe = stage_x(k + 1)
                nxt_g1 = stage_gates1(k + 1, nxt_xe)

            ENp_v, ENn_v = ENp[:], ENn[:]
            ENp_bc = ENp_v.unsqueeze(2).broadcast_to([HC, B, tb])

            # a = 1 - 0.01*Esum*exp(-n0): STT from PSUM then +1
            a = chn.tile([128, nfd], F16, tag="a")
            nc.vector.scalar_tensor_tensor(
                r3(a[:]), r3(ps_es[:]), -0.01, ENp_bc, OP.mult, OP.mult
            )
            nc.vector.tensor_scalar(a[:], a[:], 1.0, None, OP.add)
            nc.vector.tensor_scalar(
                r3(rst[:])[:, :, 0], r3(a[:])[:, :, 0], -1.0, -0.03,
                OP.mult, OP.add,
            )
            nc.vector.memset(r3(a[:])[:, :, 0], 0.0)
            dt = chn.tile([128, nfd], F16, tag="dt")
            nc.vector.tensor_tensor_scan(
                dt[:], a[:], rst[:], 0.0, OP.mult, OP.add
            )

            # ENd = -e^{-n0}*dt_{t-1} (pre-update carries!), col0 = +e^{-n0}
            Eend = sm.tile([HC, B], F32, tag="Eend")
            nc.scalar.activation(
                Eend[:], r3(dt[:])[:, :, tb - 1], AF.Exp, bias=bm1[:],
                scale=-1.0,
            )
            ENd = chn.tile([128, nfd], F16, tag="ENd")
            nc.vector.tensor_mul(
                r3(ENd[:])[:, :, 1:tb],
                ENn_v.unsqueeze(2).broadcast_to([HC, B, tb - 1]),
                r3(dt[:])[:, :, 0 : tb - 1],
            )
            nc.vector.tensor_copy(r3(ENd[:])[:, :, 0], ENp_v)
            # EN carry update
            nc.vector.tensor_mul(EN32[:], EN32[:], Eend[:])
            nc.scalar.copy(ENp[:], EN32[:])
            nc.scalar.mul(ENn[:], EN32[:], -1.0)

            if have_next:
                nxt_g2 = stage_gates2(k + 1, nxt_xe)

            # c-scan operands: fc = Ef*ENd (in Ef), ic = Ei*ENd*g (in Ei)
            nc.vector.tensor_mul(Ef[:], Ef[:], ENd[:])
            nc.vector.tensor_mul(Ei[:], Ei[:], ENd[:])
            nc.vector.tensor_mul(Ei[:], Ei[:], Tg[:])
            t64 = sm.tile([HC, B], F16, tag="t64")
            nc.vector.tensor_mul(t64[:], r3(Ef[:])[:, :, 0], Cc_v)
            nc.vector.tensor_add(
                r3(Ei[:])[:, :, 0], r3(Ei[:])[:, :, 0], t64[:]
            )
            nc.vector.memset(r3(Ef[:])[:, :, 0], 0.0)
            c = chn.tile([128, nfd], F16, tag="c")
            nc.vector.tensor_tensor_scan(c[:], Ef[:], Ei[:], 0.0, OP.mult, OP.add)
            Ccl = sm.tile([HC, B], F16, tag="ccl")
            nc.vector.tensor_scalar_min(Ccl[:], r3(c[:])[:, :, tb - 1], CCLAMP)

            if have_next:
                nxt_es = stage_esum(k + 1, nxt_xe, nxt_g1[0], nxt_g1[1], nxt_g1[2])
                nxt_l1 = stage_l1(k + 1, nxt_g2[1])

            # sigmoid(c) via tanh; Tc+1 as a second ACT pass
            Tc = chn.tile([128, nfd], F16, tag="Tc")
            nc.scalar.activation(Tc[:], c[:], AF.Tanh, scale=0.5)
            nc.scalar.activation(Tc[:], Tc[:], AF.Identity, bias=bp1[:])

            # h input: bh = Eo*ENd*L1D*(Tc+1)  (in Eo)
            nc.vector.tensor_mul(Eo[:], Eo[:], ENd[:])
            L1D = chn.tile([128, nfd], F16, tag="L1D")
            nc.scalar.activation(
                L1D[:], Sq[:], AF.Identity, bias=bld[:], scale=DT / 2
            )
            nc.vector.tensor_mul(Eo[:], Eo[:], L1D[:])
            nc.vector.tensor_mul(Eo[:], Eo[:], Tc[:])
            t64b = sm.tile([HC, B], F32, tag="t64b")
            nc.vector.tensor_mul(t64b[:], r3(L1[:])[:, :, 0], Hc_v)
            nc.vector.tensor_add(
                r3(Eo[:])[:, :, 0], r3(Eo[:])[:, :, 0], t64b[:]
            )
            nc.vector.memset(r3(L1[:])[:, :, 0], 0.0)
            h = chn.tile([128, nfd], F16, tag="h")
            nc.vector.tensor_tensor_scan(h[:], L1[:], Eo[:], 0.0, OP.mult, OP.add)

            # y partials
            ps = pp.tile([128, tb], F32, tag="y")
            for j in range(nslab):
                nc.tensor.matmul(
                    ps[:, 2 * j : 2 * j + 2],
                    h[:, 128 * j : 128 * (j + 1)],
                    pj[:],
                    start=True,
                    stop=True,
                )
            ysb = sm.tile([128, tb], F32, tag="ysb")
            nc.scalar.copy(ysb[:], ps[:])
            nc.sync.dma_start(y_d[k], ysb[:])

            Cc_v = Ccl[:]
            Hc_v = r3(h[:])[:, :, tb - 1]
            if have_next:
                tiles = (nxt_g1[0], nxt_g1[1], nxt_g1[2], nxt_g1[3],
                         nxt_g2[0], nxt_g2[1], nxt_l1, nxt_es)

    nc.compile()
    return nc


def _get_program():
    key = (S, TB)
    if key not in _cached:
        _cached[key] = build_program(S, TB)
    return _cached[key]


def host_inputs(x_codes, Wi_w, Wi_b, Wf_w, Wf_b, Wo_w, Wo_b, Wg_w, Wg_b,
                Wl_w, Wl_b, proj_w, proj_b, n_init):
    """Fold input normalization into per-gate ACT scale/bias; shard over H."""
    f = lambda v: np.asarray(v, np.float32)
    cols = []
    for (w, b) in ((Wi_w, Wi_b), (Wf_w, Wf_b), (Wo_w, Wo_b)):
        cols += [f(w) / 100.0, f(b) - 0.65 * f(w)]
    for (w, b) in ((Wg_w, Wg_b), (Wl_w, Wl_b)):
        cols += [f(w) / 200.0, (f(b) - 0.65 * f(w)) / 2.0]
    wv_full = np.stack(cols, axis=1).astype(np.float32)  # [H, 10]
    x = np.ascontiguousarray(f(x_codes)).astype(np.float16)
    pw = f(proj_w)
    en0 = np.exp(-f(n_init))
    ident = np.eye(128, dtype=np.float16)
    maps = []
    for k in range(NCORES):
        hs = slice(k * HC, (k + 1) * HC)
        maps.append({
            "x": x,
            "wv": np.ascontiguousarray(wv_full[hs]),
            "projT": np.ascontiguousarray(pw[:, hs].T.astype(np.float16)),
            "en0": np.ascontiguousarray(en0[hs].reshape(HC, 1)),
            "ident": ident,
        })
    return maps


def assemble_output(results, proj_b, s=S, tb=TB):
    nb = s // tb
    nslab = (B * tb) // 128
    bper = 128 // tb
    y = np.zeros((B, s, 2), np.float64)
    for k in range(NCORES):
        yc = np.asarray(results[k]["yout"], np.float64)
        ycr = yc.reshape(nb, bper, tb, nslab, 2)
        y += np.transpose(ycr, (3, 1, 0, 2, 4)).reshape(B, s, 2)
    y += np.asarray(proj_b, np.float64)[None, None, :]
    return y.astype(np.float32)


def kernel(**inputs):
    global _last_results
    nc = _get_program()
    maps = host_inputs(**inputs)
    res = run_bass_kernel_spmd(
        nc, maps, list(range(NCORES)),
        trace=bool(os.environ.get("KTRACE")),
        tmpdir=os.environ.get("KTRACE_DIR") or None,
    )
    _last_results = res
    return assemble_output(res.results, inputs["proj_b"])
